# revision 1
# baseline (speedup 1.0000x reference)
"""Conditional BatchNorm1d (training-mode, per-class stats) on 8 Trainium2
NeuronCores.

Problem: x [512, 128, 1024] f32, labels [512] i32 in [0,8), weight/bias
[8, 128] f32.  Per-class biased mean/var over the class's (batch, length)
elements per feature, then per-class affine:
    y = x * (rsqrt(var+eps)*w)[lbl] + (b - mean*rsqrt(var+eps)*w)[lbl]

Sharding: data-parallel over batch B across the 8 cores (64 batches each).
Each core streams its x shard once to accumulate per-(class, feature)
sum / sum-of-squares, the tiny [16, 128] partials are AllReduced on-device,
scale/shift are computed and gathered per batch with small matmuls against
the one-hot label mask, and a second streaming pass applies the affine.

Layout: the host hands each core its shard transposed to feature-major
[F=128, B_LOC=64, L=1024] so a single DMA can move a 2-batch group with
8 KiB of DRAM-contiguous data per partition (measured ~390 GB/s vs
~360 GB/s for per-batch 4 KiB lines).  The tail RES batches of pass 1 stay
resident in SBUF, so pass 2 only re-reads the first B_LOC-RES batches.
"""

import sys

if "/opt/trn_rl_repo" not in sys.path:
    sys.path.insert(0, "/opt/trn_rl_repo")

import numpy as np

import concourse.bacc as bacc
import concourse.tile as tile
from concourse import mybir
from concourse import bass_utils

B, F, L = 512, 128, 1024
K = 8
N_CORES = 8
B_LOC = B // N_CORES  # 64
EPS = 1e-5
GRP = 2               # batches per DMA group
RES = 20              # resident batches (multiple of GRP)

F32 = mybir.dt.float32
AFT = mybir.ActivationFunctionType

_built = None


def _build():
    nc = bacc.Bacc("TRN2", target_bir_lowering=False, debug=False,
                   num_devices=N_CORES)

    x = nc.dram_tensor("x", [F, B_LOC, L], F32, kind="ExternalInput")
    # One-hot label mask, transposed: maskT[k, j] = 1 iff labels[shard j] == k
    maskT = nc.dram_tensor("maskT", [K, B_LOC], F32, kind="ExternalInput")
    # Block-diagonal mask for the stats matmul: mask2[j, k] = maskT[k, j] and
    # mask2[64+j, 8+k] = maskT[k, j] (sum half / sum-of-squares half).
    mask2 = nc.dram_tensor("mask2", [2 * B_LOC, 2 * K], F32,
                           kind="ExternalInput")
    ident = nc.dram_tensor("ident", [128, 128], F32, kind="ExternalInput")
    rcp_cnt = nc.dram_tensor("rcp_cnt", [K, 1], F32, kind="ExternalInput")
    epsv = nc.dram_tensor("epsv", [K, 1], F32, kind="ExternalInput")
    weight = nc.dram_tensor("weight", [K, F], F32, kind="ExternalInput")
    bias = nc.dram_tensor("bias", [K, F], F32, kind="ExternalInput")
    y = nc.dram_tensor("y", [F, B_LOC, L], F32, kind="ExternalOutput")

    n_grp = B_LOC // GRP
    res_grp = RES // GRP
    stream_grp = n_grp - res_grp   # groups re-read in pass 2

    with tile.TileContext(nc) as tc:
        with (
            tc.tile_pool(name="const", bufs=1) as constp,
            tc.tile_pool(name="xin", bufs=5) as xin,
            tc.tile_pool(name="xres", bufs=res_grp) as xres,
            tc.tile_pool(name="stats", bufs=1) as statsp,
            tc.tile_pool(name="psum", bufs=1, space="PSUM") as psum,
            tc.tile_pool(name="dram", bufs=1, space="DRAM") as dram,
            tc.tile_pool(name="xin2", bufs=6) as xin2,
            tc.tile_pool(name="yout", bufs=3) as yout,
        ):
            # const loads issue from the ACT sequencer so the first x loads
            # lead the in-order Sync stream.  Consts are packed into two
            # tiles: every tile burns a 4KB/partition slot regardless of
            # size, so separate tiny tiles would waste ~24KB/partition.
            cpack1 = constp.tile([128, 144], F32)
            identt = cpack1[:, 0:128]
            mask2t = cpack1[:, 128:144]
            nc.scalar.dma_start(identt, ident[:])
            nc.scalar.dma_start(mask2t, mask2[:])
            cpack2 = constp.tile([K, 322], F32)
            maskTt = cpack2[:, 0:B_LOC]
            rcpt = cpack2[:, B_LOC:B_LOC + 1]
            epst = cpack2[:, B_LOC + 1:B_LOC + 2]
            wt = cpack2[:, 66:194]
            bt = cpack2[:, 194:322]
            nc.scalar.dma_start(maskTt, maskT[:])
            nc.scalar.dma_start(rcpt, rcp_cnt[:])
            nc.scalar.dma_start(epst, epsv[:])
            nc.scalar.dma_start(wt, weight[:])
            nc.scalar.dma_start(bt, bias[:])

            # ---- pass 1: per-batch row sums / sums of squares ----
            # S[:, b] = sum_l x[:, b, l] (DVE); Q[:, b] = sum_l x[:, b, l]^2
            # (ACT).  Separate S/Q tiles: a shared tile would make Tile
            # serialize the two engines on false WAW sharing.
            # Packed stat tiles (slot economy); S and Q stay in separate
            # tiles so ACT and DVE never false-share a written tile in the
            # hot loop.  spackA is all-DVE-written, spackB all-ACT + late
            # DVE, spackC holds the small serial chain.
            spackA = statsp.tile([128, 256], F32)
            S = spackA[:, 0:B_LOC]
            sqt = spackA[:, 64:192]
            ssel = spackA[:, 192:256]
            spackB = statsp.tile([128, 128], F32)
            Q = spackB[:, 0:B_LOC]
            tsel = spackB[:, 64:128]
            spackC = statsp.tile([2 * K, 1280], F32)
            part = spackC[:, 0:128]
            Gs = spackC[0:K, 128:256]
            Gq = spackC[0:K, 256:384]
            chain = spackC[0:K, 384:1280]
            # ACT square scratch lives in PSUM (2 banks), saving SBUF
            scratch_a = psum.tile([128, L], F32)
            res_tiles = {}
            for g in range(n_grp):
                if g >= stream_grp:
                    xt = xres.tile([F, GRP * L], F32)
                    res_tiles[g] = xt
                else:
                    xt = xin.tile([F, GRP * L], F32, tag="xs")
                nc.sync.dma_start(xt[:], x[:, g * GRP:(g + 1) * GRP, :])
                for i in range(GRP):
                    b = g * GRP + i
                    xs = xt[:, i * L:(i + 1) * L]
                    nc.scalar.activation(scratch_a[:], xs, AFT.Square,
                                         accum_out=Q[:, b:b + 1])
                    nc.vector.reduce_sum(S[:, b:b + 1], xs,
                                         axis=mybir.AxisListType.X)

            # ---- per-class reduction: transpose + masked matmul ----
            # sqt partitions 0..63 = S^T (batch-major), 64..127 = Q^T.
            st_ps = psum.tile([B_LOC, 128], F32)
            nc.tensor.transpose(st_ps[:], S, identt)
            qt_ps = psum.tile([B_LOC, 128], F32)
            nc.tensor.transpose(qt_ps[:], Q, identt)
            nc.vector.tensor_copy(spackA[0:B_LOC, 64:192], st_ps[:])
            nc.vector.tensor_copy(spackA[B_LOC:128, 64:192], qt_ps[:])

            part_ps = psum.tile([2 * K, 128], F32)
            nc.tensor.matmul(part_ps[:], mask2t, sqt, start=True,
                             stop=True)
            nc.vector.tensor_copy(part, part_ps[:])

            # ---- all-reduce the [16, 128] partials across the 8 cores ----
            cc_in = dram.tile([2 * K, 128], F32)
            cc_out = dram.tile([2 * K, 128], F32)
            # upload via GpSimd: it waits on `part`, and a wait on the
            # in-order Sync stream would stall the pass-2 prefetch issues
            nc.gpsimd.dma_start(cc_in[:], part)
            nc.gpsimd.collective_compute(
                "AllReduce",
                mybir.AluOpType.add,
                replica_groups=[list(range(N_CORES))],
                ins=[cc_in.opt()],
                outs=[cc_out.opt()],
            )
            # G loads issue from the ACT sequencer: they must wait for the
            # AllReduce, and a wait on the in-order Sync stream would block
            # the pass-2 prefetch issues queued behind it.
            nc.scalar.dma_start(Gs, cc_out[0:K])
            nc.scalar.dma_start(Gq, cc_out[K:2 * K])

            # ---- scale/shift per (class, feature) ----
            mean = chain[:, 0 * F:1 * F]
            msq = chain[:, 1 * F:2 * F]
            var = chain[:, 2 * F:3 * F]
            std = chain[:, 3 * F:4 * F]
            inv = chain[:, 4 * F:5 * F]
            scal = chain[:, 5 * F:6 * F]
            shft = chain[:, 6 * F:7 * F]
            nc.vector.tensor_scalar_mul(mean, Gs, rcpt)
            nc.vector.tensor_scalar_mul(msq, Gq, rcpt)
            nc.vector.tensor_mul(var, mean, mean)
            nc.vector.tensor_sub(var, msq, var)
            nc.scalar.activation(std, var, AFT.Sqrt, bias=epst)
            nc.vector.reciprocal(inv, std)
            nc.vector.tensor_mul(scal, inv, wt)
            nc.vector.tensor_mul(shft, mean, scal)
            nc.vector.tensor_sub(shft, bt, shft)

            # ---- select per-batch scale/shift columns: [F, B_LOC] ----
            ssel_ps = psum.tile([F, B_LOC], F32)
            nc.tensor.matmul(ssel_ps[:], scal, maskTt, start=True,
                             stop=True)
            nc.vector.tensor_copy(ssel, ssel_ps[:])
            tsel_ps = psum.tile([F, B_LOC], F32)
            nc.tensor.matmul(tsel_ps[:], shft, maskTt, start=True,
                             stop=True)
            nc.vector.tensor_copy(tsel, tsel_ps[:])

            # ---- pass 2: y[:, b] = x[:, b] * ssel[:, b] + tsel[:, b] ----
            # Whole group handled by one engine (group parity): keeps the
            # ACT/DVE streams independent, no shared-tile serialization.
            # Resident groups first: their applies are ready the moment
            # ssel/tsel land, keeping stores busy while reloads stream.
            def apply_group(g, xt_tile):
                yt = yout.tile([F, GRP * L], F32)
                for i in range(GRP):
                    b = g * GRP + i
                    xs = xt_tile[:, i * L:(i + 1) * L]
                    ys = yt[:, i * L:(i + 1) * L]
                    if g % 2 == 0:
                        nc.scalar.activation(ys, xs, AFT.Identity,
                                             bias=tsel[:, b:b + 1],
                                             scale=ssel[:, b:b + 1])
                    else:
                        nc.vector.tensor_scalar(ys, xs,
                                                ssel[:, b:b + 1],
                                                tsel[:, b:b + 1],
                                                mybir.AluOpType.mult,
                                                mybir.AluOpType.add)
                nc.gpsimd.dma_start(y[:, g * GRP:(g + 1) * GRP, :], yt[:])

            # Interleave resident and streamed groups: resident applies are
            # ready the instant ssel/tsel land (stores start immediately),
            # while early streamed applies free load slots so the reload
            # stream never waits behind a block of resident-only work.
            for j in range(max(res_grp, stream_grp)):
                if j < res_grp:
                    apply_group(stream_grp + j, res_tiles[stream_grp + j])
                if j < stream_grp:
                    # First reloads reuse the freed pass-1 xin slots: deeper
                    # prefetch over the AllReduce window at no SBUF cost.
                    pool = xin if j < 5 else xin2
                    xt2 = pool.tile([F, GRP * L], F32, tag="xs")
                    nc.sync.dma_start(xt2[:], x[:, j * GRP:(j + 1) * GRP, :])
                    apply_group(j, xt2)

    nc.finalize()
    return nc


def _get_nc():
    global _built
    if _built is None:
        _built = _build()
    return _built


def _host_inputs(x, labels, weight, bias):
    labels = np.asarray(labels).astype(np.int64)
    counts = np.bincount(labels, minlength=K).astype(np.float64) * L
    rcp = (1.0 / np.maximum(counts, 1.0)).astype(np.float32).reshape(K, 1)
    ident = np.eye(128, dtype=np.float32)

    in_maps = []
    for c in range(N_CORES):
        lab = labels[c * B_LOC:(c + 1) * B_LOC]
        maskT = np.zeros((K, B_LOC), dtype=np.float32)
        maskT[lab, np.arange(B_LOC)] = 1.0
        mask2 = np.zeros((2 * B_LOC, 2 * K), dtype=np.float32)
        mask2[:B_LOC, :K] = maskT.T
        mask2[B_LOC:, K:] = maskT.T
        in_maps.append({
            # feature-major shard: [F, B_LOC, L]
            "x": np.ascontiguousarray(
                x[c * B_LOC:(c + 1) * B_LOC].transpose(1, 0, 2)),
            "maskT": maskT,
            "mask2": mask2,
            "ident": ident,
            "rcp_cnt": rcp,
            "epsv": np.full((K, 1), EPS, dtype=np.float32),
            "weight": np.ascontiguousarray(weight.astype(np.float32)),
            "bias": np.ascontiguousarray(bias.astype(np.float32)),
        })
    return in_maps


def run(x, labels, weight, bias, trace=False):
    nc = _get_nc()
    in_maps = _host_inputs(x, labels, weight, bias)
    res = bass_utils.run_bass_kernel_spmd(nc, in_maps, list(range(N_CORES)),
                                          trace=trace)
    out = np.concatenate(
        [res.results[c]["y"].transpose(1, 0, 2) for c in range(N_CORES)],
        axis=0)
    return out, res


def kernel(x, labels, weight, bias):
    out, _ = run(np.asarray(x, dtype=np.float32), labels,
                 np.asarray(weight, dtype=np.float32),
                 np.asarray(bias, dtype=np.float32))
    return out



# revision 23
# speedup vs baseline: 2.6674x; 2.6674x over previous
"""Conditional BatchNorm1d (training-mode, per-class stats) on 8 Trainium2
NeuronCores.

Problem: x [512, 128, 1024] f32, labels [512] i32 in [0,8), weight/bias
[8, 128] f32.  Per-class biased mean/var over the class's (batch, length)
elements per feature, then per-class affine:
    y = x * (rsqrt(var+eps)*w)[lbl] + (b - mean*rsqrt(var+eps)*w)[lbl]

Sharding: FEATURE-parallel across the 8 cores (16 features each, all 512
batches).  Per-class statistics only mix (batch, length) elements of the
SAME feature, so each core owns its features' stats outright — no
AllReduce at all (the B-sharded variant spent ~33 us in one).

Precision: the host hands each core its x shard pre-cast to bf16 and takes
y back in bf16 (rel err ~2e-3, an order under the 2e-2 gate).  That halves
HBM traffic to 16 MiB in + 16 MiB out per core, and the whole 16 MiB shard
stays resident in SBUF so x is read exactly once.

Engine split (per-feature op costs measured on HW):
  - class SUMS go to the otherwise-idle PE as 128 accumulating matmuls:
    stationary = per-(block, feature) one-hot masks expanded to [128, 128]
    fp8 with the 8 mask columns at feature-specific row octets, so all 16
    features accumulate into the same two [128, 512] PSUM banks (matmul
    output must fit one bank) and the drain is two full-width DVE reduces.
    The [128]-partition (f, k) drain bounces through DRAM to come back
    k-major for the scale/shift chain.
  - SUMS OF SQUARES split DVE (custom-DVE affine_mul_reduce, x*(x*1+0)
    fused square+reduce) / ACT (Square + accumulator), both ~1.2 us per
    feature; DVE reduce/accum ops don't get the 2x bf16 mode.
  - pass 2 applies split DVE tensor_scalar (2x bf16, 475 ns) / ACT
    activation Identity (1.2 us).
"""

import sys

if "/opt/trn_rl_repo" not in sys.path:
    sys.path.insert(0, "/opt/trn_rl_repo")

import ml_dtypes
import numpy as np

import concourse.bacc as bacc
import concourse.tile as tile
from concourse import mybir
from concourse import bass_utils

B, F, L = 512, 128, 1024
K = 8
N_CORES = 8
F_LOC = F // N_CORES          # 16 features per core
NB = 4                        # batch blocks of 128
BB = B // NB                  # 128 batches per block
NQ = 4                        # feature quads per core
FQ = F_LOC // NQ              # 4 features per quad
HL = L // 2                   # matmul output must fit one PSUM bank
EPS = 1e-5

# within each quad, features j<2 take the DVE square path
# (affine_mul_reduce) and j>=2 the ACT Square+accum path, so both engines
# have work from the first tile onward.
N_DVE_F = 2 * NQ
N_ACT_F = F_LOC - N_DVE_F

F32 = mybir.dt.float32
BF16 = mybir.dt.bfloat16
FP8 = mybir.dt.float8e4
AFT = mybir.ActivationFunctionType
ALU = mybir.AluOpType

_built = None


def _build():
    nc = bacc.Bacc("TRN2", target_bir_lowering=False, debug=False,
                   num_devices=N_CORES)

    x = nc.dram_tensor("x", [B, F_LOC, L], BF16, kind="ExternalInput")
    # mexp[p, (bb*16+f)*128 + 8*f + k] = 1 iff labels[bb*128+p] == k:
    # expanded one-hot masks, mask block at row-octet f so every feature's
    # PE class-sum accumulates into a distinct octet of one PSUM bank.
    mexp = nc.dram_tensor("mexp", [BB, NB * F_LOC * BB], FP8,
                          kind="ExternalInput")
    # maskb[p, bb*8 + k] = 1 iff labels[bb*128 + p] == k (Q class-reduce),
    # then E16[p, f] = (p//8 == f) and P8[p, k] = (p%8 == k) for the
    # on-chip (f,k)-partition -> k-major rearrange of the PE sum drain.
    maskb = nc.dram_tensor("maskb", [BB, NB * K + F_LOC + K], F32,
                           kind="ExternalInput")
    # maskTb[k, bb*128 + p] = same, transposed for the select matmuls
    maskTb = nc.dram_tensor("maskTb", [K, B], F32, kind="ExternalInput")
    rcp_cnt = nc.dram_tensor("rcp_cnt", [K, 1], F32, kind="ExternalInput")
    epsv = nc.dram_tensor("epsv", [K, 1], F32, kind="ExternalInput")
    wloc = nc.dram_tensor("wloc", [K, F_LOC], F32, kind="ExternalInput")
    bloc = nc.dram_tensor("bloc", [K, F_LOC], F32, kind="ExternalInput")
    y = nc.dram_tensor("y", [B, F_LOC, L], BF16, kind="ExternalOutput")

    with tile.TileContext(nc) as tc:
        with (
            tc.tile_pool(name="const", bufs=1) as constp,
            tc.tile_pool(name="stats", bufs=1) as statsp,
            tc.tile_pool(name="xres", bufs=NB * NQ) as xres,
            tc.tile_pool(name="psum", bufs=1, space="PSUM") as psum,
            tc.tile_pool(name="yout", bufs=5) as yout,
        ):
            # ---- consts (ACT sequencer so x loads lead the Sync stream) ----
            mexpt = constp.tile([BB, NB * F_LOC * BB], FP8)
            nc.scalar.dma_start(mexpt[:], mexp[:])
            maskbt = constp.tile([BB, NB * K + F_LOC + K], F32)
            nc.scalar.dma_start(maskbt[:], maskb[:])
            e16t = maskbt[:, NB * K:NB * K + F_LOC]
            p8t = maskbt[:, NB * K + F_LOC:NB * K + F_LOC + K]
            cpack = constp.tile([K, B + 2 + 2 * F_LOC], F32)
            maskTt = cpack[:, 0:B]
            rcpt = cpack[:, B:B + 1]
            epst = cpack[:, B + 1:B + 2]
            wt = cpack[:, B + 2:B + 2 + F_LOC]
            bt = cpack[:, B + 2 + F_LOC:B + 2 + 2 * F_LOC]
            nc.scalar.dma_start(maskTt, maskTb[:])
            nc.scalar.dma_start(rcpt, rcp_cnt[:])
            nc.scalar.dma_start(epst, epsv[:])
            nc.scalar.dma_start(wt, wloc[:])
            nc.scalar.dma_start(bt, bloc[:])

            # ---- stat tiles (single writer engine per tile) ----
            Qd = statsp.tile([BB, NB * N_DVE_F], F32)     # DVE sumsq
            Qa = statsp.tile([BB, NB * N_ACT_F], F32)     # ACT sumsq
            dscr = statsp.tile([BB, L], BF16)             # DVE amr out scratch
            ascr = statsp.tile([BB, L], BF16)             # ACT square scratch
            sel = statsp.tile([BB, NB * 2 * F_LOC], F32)  # scale/shift per b
            chain = statsp.tile([K, 12 * F_LOC], F32)
            Dt = statsp.tile([BB, 3 + F_LOC], F32)        # PE sum drains

            pst = psum.tile([BB, 2, HL], F32)             # PE class sums

            # dummy Sqrt first on ACT: pulls in the sqrt_and_others table
            # (which also holds square/identity/copy) during load latency,
            # so the scale/shift chain later needs no mid-kernel table swap.
            nc.scalar.activation(ascr[0:K, 0:1], epst, AFT.Sqrt)

            # ---- pass 1: stream shard; PE sums + DVE/ACT sums of squares --
            res_tiles = {}
            n_tiles = NB * NQ
            ti = 0
            for bb in range(NB):
                for q in range(NQ):
                    xt = xres.tile([BB, FQ, L], BF16, tag="xs")
                    res_tiles[(bb, q)] = xt
                    nc.sync.dma_start(
                        xt[:], x[bb * BB:(bb + 1) * BB, q * FQ:(q + 1) * FQ, :])
                    st = (ti == 0)
                    sp = (ti == n_tiles - 1)
                    for j in range(FQ):
                        f = q * FQ + j
                        lhs = mexpt[:, (bb * F_LOC + f) * BB:
                                    (bb * F_LOC + f + 1) * BB]
                        nc.tensor.matmul(pst[:, 0, :], lhs, xt[:, j, 0:HL],
                                         start=st and j == 0,
                                         stop=sp and j == FQ - 1,
                                         skip_group_check=True)
                        nc.tensor.matmul(pst[:, 1, :], lhs, xt[:, j, HL:L],
                                         start=st and j == 0,
                                         stop=sp and j == FQ - 1,
                                         skip_group_check=True)
                        if j < 2:
                            c = bb * N_DVE_F + 2 * q + j
                            nc.vector.affine_mul_reduce(
                                dscr[:], Qd[:, c:c + 1],
                                xt[:, j], xt[:, j], 1.0, 0.0)
                        else:
                            c = bb * N_ACT_F + 2 * q + (j - 2)
                            nc.scalar.activation(
                                ascr[:], xt[:, j], AFT.Square,
                                accum_out=Qa[:, c:c + 1])
                    ti += 1

            # ---- drain PE sums: [128, 1] over (f, k), rearrange k-major ----
            # rcp(count) is folded into P8 and maskb on the host, so the
            # rearrange matmul emits MEAN and the Q matmuls emit E[x^2].
            nc.vector.reduce_sum(Dt[:, 2:3], pst[:],
                                 axis=mybir.AxisListType.XY)
            Bx = Dt[:, 3:3 + F_LOC]
            nc.vector.tensor_scalar(Bx, e16t, Dt[:, 2:3], 0.0,
                                    ALU.mult, ALU.add)
            mean_ps = psum.tile([K, F_LOC], F32)
            nc.tensor.matmul(mean_ps[:], p8t, Bx, start=True, stop=True)

            # ---- Q class-reduce: compact [8, 8] psums per square path ----
            ps_Qd = psum.tile([K, N_DVE_F], F32)
            ps_Qa = psum.tile([K, N_ACT_F], F32)
            for bb in range(NB):
                mk = maskbt[:, bb * K:(bb + 1) * K]
                st = (bb == 0)
                sp = (bb == NB - 1)
                nc.tensor.matmul(ps_Qd[:], mk,
                                 Qd[:, bb * N_DVE_F:(bb + 1) * N_DVE_F],
                                 start=st, stop=sp)
                nc.tensor.matmul(ps_Qa[:], mk,
                                 Qa[:, bb * N_ACT_F:(bb + 1) * N_ACT_F],
                                 start=st, stop=sp)

            # ---- scale/shift chain, k-major on [8, 16] ----
            # tmp3/var3 are (q, j)-shaped so the compact Qd/Qa psums land
            # at their interleaved feature columns via strided DVE outs.
            tmp3 = statsp.tile([K, NQ, FQ], F32)
            var3 = statsp.tile([K, NQ, FQ], F32)
            tmp = chain[:, 4 * F_LOC:5 * F_LOC]
            std = chain[:, 6 * F_LOC:7 * F_LOC]
            inv = chain[:, 7 * F_LOC:8 * F_LOC]
            scal = chain[:, 8 * F_LOC:9 * F_LOC]
            shft = chain[:, 9 * F_LOC:10 * F_LOC]
            nc.scalar.activation(tmp3[:], mean_ps[:], AFT.Square)
            nc.vector.tensor_sub(var3[:, :, 0:2], ps_Qd[:], tmp3[:, :, 0:2])
            nc.vector.tensor_sub(var3[:, :, 2:4], ps_Qa[:], tmp3[:, :, 2:4])
            nc.scalar.activation(std, var3[:], AFT.Sqrt, bias=epst)
            nc.vector.reciprocal(inv, std)
            nc.vector.tensor_mul(scal, inv, wt)
            nc.vector.tensor_mul(tmp, mean_ps[:], scal)
            nc.vector.tensor_sub(shft, bt, tmp)

            # ---- select per-batch scale/shift: [128, 32] per block ----
            selps0 = psum.tile([BB, 2 * F_LOC], F32)
            selps1 = psum.tile([BB, 2 * F_LOC], F32)
            for bb in range(NB):
                sp = selps0 if bb % 2 == 0 else selps1
                nc.tensor.matmul(sp[:], maskTt[:, bb * BB:(bb + 1) * BB],
                                 chain[:, 8 * F_LOC:10 * F_LOC],
                                 start=True, stop=True)
                nc.vector.tensor_copy(
                    sel[:, bb * 2 * F_LOC:(bb + 1) * 2 * F_LOC], sp[:])

            # ---- pass 2: y = x*scale + shift from resident tiles ----
            op_idx = 0
            for bb in range(NB):
                for q in range(NQ):
                    xt = res_tiles[(bb, q)]
                    yt = yout.tile([BB, FQ, L], BF16, tag="ys")
                    for j in range(FQ):
                        f = q * FQ + j
                        sc = sel[:, bb * 2 * F_LOC + f:
                                 bb * 2 * F_LOC + f + 1]
                        sh = sel[:, bb * 2 * F_LOC + F_LOC + f:
                                 bb * 2 * F_LOC + F_LOC + f + 1]
                        if op_idx % 4 == 3:
                            nc.scalar.activation(yt[:, j], xt[:, j],
                                                 AFT.Identity,
                                                 bias=sh, scale=sc)
                        else:
                            nc.vector.tensor_scalar(yt[:, j], xt[:, j],
                                                    sc, sh,
                                                    ALU.mult, ALU.add)
                        op_idx += 1
                    nc.gpsimd.dma_start(
                        y[bb * BB:(bb + 1) * BB, q * FQ:(q + 1) * FQ, :],
                        yt[:])

    nc.finalize()
    return nc


def _get_nc():
    global _built
    if _built is None:
        _built = _build()
    return _built


def _host_inputs(x, labels, weight, bias):
    labels = np.asarray(labels).astype(np.int64)
    counts = np.bincount(labels, minlength=K).astype(np.float64) * L
    rcp = (1.0 / np.maximum(counts, 1.0)).astype(np.float32).reshape(K, 1)

    maskb = np.zeros((BB, NB * K + F_LOC + K), dtype=np.float32)
    maskTb = np.zeros((K, B), dtype=np.float32)
    mexp = np.zeros((BB, NB * F_LOC * BB), dtype=np.float32)
    p = np.arange(BB)
    rcpf = rcp.reshape(K)
    for bb in range(NB):
        lab = labels[bb * BB:(bb + 1) * BB]
        maskb[p, bb * K + lab] = rcpf[lab]       # rcp folded: Q matmul -> E[x^2]
        for f in range(F_LOC):
            mexp[p, (bb * F_LOC + f) * BB + 8 * f + lab] = 1.0
    maskb[p, NB * K + p // K] = 1.0              # E16: p//8 == f
    maskb[p, NB * K + F_LOC + p % K] = rcpf[p % K]   # P8*rcp: rearrange -> mean
    maskTb[labels, np.arange(B)] = 1.0
    mexp8 = mexp.astype(mybir.dt.np(FP8))

    xb = np.ascontiguousarray(x).astype(ml_dtypes.bfloat16)
    in_maps = []
    for c in range(N_CORES):
        in_maps.append({
            "x": np.ascontiguousarray(xb[:, c * F_LOC:(c + 1) * F_LOC, :]),
            "mexp": mexp8,
            "maskb": maskb,
            "maskTb": maskTb,
            "rcp_cnt": rcp,
            "epsv": np.full((K, 1), EPS, dtype=np.float32),
            "wloc": np.ascontiguousarray(
                weight[:, c * F_LOC:(c + 1) * F_LOC].astype(np.float32)),
            "bloc": np.ascontiguousarray(
                bias[:, c * F_LOC:(c + 1) * F_LOC].astype(np.float32)),
        })
    return in_maps


def run(x, labels, weight, bias, trace=False):
    nc = _get_nc()
    in_maps = _host_inputs(x, labels, weight, bias)
    res = bass_utils.run_bass_kernel_spmd(nc, in_maps, list(range(N_CORES)),
                                          trace=trace)
    out = np.concatenate(
        [res.results[c]["y"] for c in range(N_CORES)],
        axis=1).astype(np.float32)
    return out, res


def kernel(x, labels, weight, bias):
    out, _ = run(np.asarray(x, dtype=np.float32), labels,
                 np.asarray(weight, dtype=np.float32),
                 np.asarray(bias, dtype=np.float32))
    return out


# revision 52
# speedup vs baseline: 2.9012x; 1.0876x over previous
"""Conditional BatchNorm1d (training-mode, per-class stats) on 8 Trainium2
NeuronCores.

Problem: x [512, 128, 1024] f32, labels [512] i32 in [0,8), weight/bias
[8, 128] f32.  Per-class biased mean/var over the class's (batch, length)
elements per feature, then per-class affine:
    y = x * (rsqrt(var+eps)*w)[lbl] + (b - mean*rsqrt(var+eps)*w)[lbl]

Sharding: FEATURE-parallel across the 8 cores (16 features each, all 512
batches).  Per-class statistics only mix (batch, length) elements of the
SAME feature, so each core owns its features' stats outright — no
AllReduce at all (the B-sharded variant spent ~33 us in one).

Precision: the host hands each core its x shard pre-cast to bf16 and takes
y back in bf16 (rel err ~2e-3, an order under the 2e-2 gate).  That halves
HBM traffic to 16 MiB in + 16 MiB out per core, and the whole 16 MiB shard
stays resident in SBUF so x is read exactly once.

Engine split (per-feature op costs measured on HW):
  - class SUMS go to the otherwise-idle PE as 128 accumulating matmuls:
    stationary = per-(block, feature) one-hot masks expanded to [128, 128]
    fp8 with the 8 mask columns at feature-specific row octets, so all 16
    features accumulate into the same two [128, 512] PSUM banks (matmul
    output must fit one bank) and the drain is two full-width DVE reduces.
    The [128]-partition (f, k) drain bounces through DRAM to come back
    k-major for the scale/shift chain.
  - SUMS OF SQUARES split DVE (custom-DVE affine_mul_reduce, x*(x*1+0)
    fused square+reduce) / ACT (Square + accumulator), both ~1.2 us per
    feature; DVE reduce/accum ops don't get the 2x bf16 mode.
  - pass 2 applies split DVE tensor_scalar (2x bf16, 475 ns) / ACT
    activation Identity (1.2 us).
"""

import sys

if "/opt/trn_rl_repo" not in sys.path:
    sys.path.insert(0, "/opt/trn_rl_repo")

import ml_dtypes
import numpy as np

import concourse.bacc as bacc
import concourse.tile as tile
from concourse import mybir
from concourse import bass_utils

B, F, L = 512, 128, 1024
K = 8
N_CORES = 8
F_LOC = F // N_CORES          # 16 features per core
NB = 4                        # batch blocks of 128
BB = B // NB                  # 128 batches per block
NQ = 4                        # feature quads per core
FQ = F_LOC // NQ              # 4 features per quad
HL = L // 2                   # matmul output must fit one PSUM bank
EPS = 1e-5

# square paths per quad: j0/j1 -> DVE affine_mul_reduce; j2 -> ACT
# Square+accumulator; j3 -> ACT Square to scratch (no accumulator drain)
# with the idle PE doing the masked class-reduce of the squared scratch.
N_DVE_F = 2 * NQ   # j0, j1
N_ACT_F = NQ       # j2

F32 = mybir.dt.float32
BF16 = mybir.dt.bfloat16
FP8 = mybir.dt.float8e4
AFT = mybir.ActivationFunctionType
ALU = mybir.AluOpType

_built = None


def _build():
    nc = bacc.Bacc("TRN2", target_bir_lowering=False, debug=False,
                   num_devices=N_CORES)

    x = nc.dram_tensor("x", [B, F_LOC, L], BF16, kind="ExternalInput")
    # mexp[p, (bb*16+f)*128 + 8*f + k] = 1 iff labels[bb*128+p] == k:
    # expanded one-hot masks, mask block at row-octet f so every feature's
    # PE class-sum accumulates into a distinct octet of one PSUM bank.
    mexp = nc.dram_tensor("mexp", [BB, NB * F_LOC * BB], FP8,
                          kind="ExternalInput")
    # maskb[p, bb*8 + k] = 1 iff labels[bb*128 + p] == k (Q class-reduce),
    # then E16[p, f] = (p//8 == f) and P8[p, k] = (p%8 == k) for the
    # on-chip (f,k)-partition -> k-major rearrange of the PE sum drain.
    maskb = nc.dram_tensor("maskb", [BB, NB * K + F_LOC + K], F32,
                           kind="ExternalInput")
    # maskTb[k, bb*128 + p] = same, transposed for the select matmuls
    maskTb = nc.dram_tensor("maskTb", [K, B], F32, kind="ExternalInput")
    rcp_cnt = nc.dram_tensor("rcp_cnt", [K, 1], F32, kind="ExternalInput")
    epsv = nc.dram_tensor("epsv", [K, 1], F32, kind="ExternalInput")
    wloc = nc.dram_tensor("wloc", [K, F_LOC], F32, kind="ExternalInput")
    bloc = nc.dram_tensor("bloc", [K, F_LOC], F32, kind="ExternalInput")
    y = nc.dram_tensor("y", [B, F_LOC, L], BF16, kind="ExternalOutput")

    with tile.TileContext(nc) as tc:
        with (
            tc.tile_pool(name="const", bufs=1) as constp,
            tc.tile_pool(name="stats", bufs=1) as statsp,
            tc.tile_pool(name="xres", bufs=NB * NQ) as xres,
            tc.tile_pool(name="psum", bufs=1, space="PSUM") as psum,
            tc.tile_pool(name="ascr2", bufs=3) as ascr2p,
            tc.tile_pool(name="yout", bufs=5) as yout,
        ):
            # ---- consts (ACT sequencer so x loads lead the Sync stream);
            # small packs first so the dummy Sqrt's eps operand lands
            # before the 1 MB mexp transfer ----
            cpack = constp.tile([K, B + 2 + 2 * F_LOC], F32)
            maskTt = cpack[:, 0:B]
            rcpt = cpack[:, B:B + 1]
            epst = cpack[:, B + 1:B + 2]
            wt = cpack[:, B + 2:B + 2 + F_LOC]
            bt = cpack[:, B + 2 + F_LOC:B + 2 + 2 * F_LOC]
            nc.scalar.dma_start(epst, epsv[:])
            nc.scalar.dma_start(rcpt, rcp_cnt[:])
            nc.scalar.dma_start(wt, wloc[:])
            nc.scalar.dma_start(bt, bloc[:])
            nc.scalar.dma_start(maskTt, maskTb[:])
            maskbt = constp.tile([BB, NB * K + F_LOC + K], F32)
            nc.scalar.dma_start(maskbt[:], maskb[:])
            e16t = maskbt[:, NB * K:NB * K + F_LOC]
            p8t = maskbt[:, NB * K + F_LOC:NB * K + F_LOC + K]
            mexpt = constp.tile([BB, NB * F_LOC * BB], FP8)
            nc.scalar.dma_start(mexpt[:], mexp[:])

            # ---- stat tiles (single writer engine per tile) ----
            Qd = statsp.tile([BB, NB * N_DVE_F], F32)     # DVE sumsq
            Qa = statsp.tile([BB, NB * N_ACT_F], F32)     # ACT sumsq
            dscr = statsp.tile([BB, L], BF16)             # DVE amr out scratch
            ascr = statsp.tile([BB, L], BF16)             # ACT square scratch
            sel = statsp.tile([BB, NB * 2 * F_LOC], F32)  # scale/shift per b
            chain = statsp.tile([K, 12 * F_LOC], F32)
            Dt = statsp.tile([BB, 4 + 2 * F_LOC], F32)    # PE drains + expands

            # bank 0: class sums; bank 1: class sumsq of j3 features.
            # Both L-halves accumulate into the same bank — PSUM adds them
            # and the drain reduces over l anyway.
            pboth = psum.tile([BB, 2, HL], F32)

            # dummy Sqrt first on ACT: pulls in the sqrt_and_others table
            # (which also holds square/identity/copy) during load latency,
            # so the scale/shift chain later needs no mid-kernel table swap.
            nc.scalar.activation(ascr[0:K, 0:1], epst, AFT.Sqrt)

            # ---- pass 1: stream shard; PE sums + 3-path sums of squares --
            res_tiles = {}
            n_tiles = NB * NQ
            ti = 0
            for bb in range(NB):
                for q in range(NQ):
                    xt = xres.tile([BB, FQ, L], BF16, tag="xs")
                    res_tiles[(bb, q)] = xt
                    nc.sync.dma_start(
                        xt[:], x[bb * BB:(bb + 1) * BB, q * FQ:(q + 1) * FQ, :])
                    st = (ti == 0)
                    sp = (ti == n_tiles - 1)
                    for j in range(FQ):
                        f = q * FQ + j
                        lhs = mexpt[:, (bb * F_LOC + f) * BB:
                                    (bb * F_LOC + f + 1) * BB]
                        nc.tensor.matmul(pboth[:, 0, :], lhs, xt[:, j, 0:HL],
                                         start=st and j == 0,
                                         stop=False,
                                         skip_group_check=True)
                        nc.tensor.matmul(pboth[:, 0, :], lhs, xt[:, j, HL:L],
                                         start=False,
                                         stop=sp and j == FQ - 1,
                                         skip_group_check=True)
                        if j < 2:
                            c = bb * N_DVE_F + 2 * q + j
                            nc.vector.affine_mul_reduce(
                                dscr[:], Qd[:, c:c + 1],
                                xt[:, j], xt[:, j], 1.0, 0.0)
                        elif j == 2:
                            c = bb * N_ACT_F + q
                            nc.scalar.activation(
                                ascr[:], xt[:, j], AFT.Square,
                                accum_out=Qa[:, c:c + 1])
                        else:
                            sq = ascr2p.tile([BB, L], BF16, tag="sq")
                            nc.scalar.activation(sq[:], xt[:, j], AFT.Square)
                            nc.tensor.matmul(pboth[:, 1, :], lhs,
                                             sq[:, 0:HL],
                                             start=st,
                                             stop=False,
                                             skip_group_check=True)
                            nc.tensor.matmul(pboth[:, 1, :], lhs,
                                             sq[:, HL:L],
                                             start=False,
                                             stop=sp,
                                             skip_group_check=True)
                    ti += 1

            # ---- drain PE stats: [128, 2] over (f, k), rearrange k-major --
            # rcp(count) is folded into P8 and maskb on the host, so the
            # rearrange matmuls emit MEAN / E[x^2] and the Qd/Qa matmuls
            # emit E[x^2] directly.
            nc.vector.reduce_sum(Dt[:, 0:2], pboth[:],
                                 axis=mybir.AxisListType.X)
            Bx = Dt[:, 4:4 + F_LOC]
            nc.vector.tensor_scalar(Bx, e16t, Dt[:, 0:1], 0.0,
                                    ALU.mult, ALU.add)
            Bq = Dt[:, 4 + F_LOC:4 + 2 * F_LOC]
            nc.vector.tensor_scalar(Bq, e16t, Dt[:, 1:2], 0.0,
                                    ALU.mult, ALU.add)
            mean_ps = psum.tile([K, F_LOC], F32)
            nc.tensor.matmul(mean_ps[:], p8t, Bx, start=True, stop=True)
            msqz_ps = psum.tile([K, NQ, FQ], F32)
            nc.tensor.matmul(msqz_ps[:], p8t, Bq, start=True, stop=True)

            # ---- Q class-reduce: compact psums per square path ----
            ps_Qd = psum.tile([K, N_DVE_F], F32)
            ps_Qa = psum.tile([K, N_ACT_F], F32)
            for bb in range(NB):
                mk = maskbt[:, bb * K:(bb + 1) * K]
                st = (bb == 0)
                sp = (bb == NB - 1)
                nc.tensor.matmul(ps_Qd[:], mk,
                                 Qd[:, bb * N_DVE_F:(bb + 1) * N_DVE_F],
                                 start=st, stop=sp)
                nc.tensor.matmul(ps_Qa[:], mk,
                                 Qa[:, bb * N_ACT_F:(bb + 1) * N_ACT_F],
                                 start=st, stop=sp)

            # ---- scale/shift chain, k-major on [8, 16] ----
            # tmp3/var3 are (q, j)-shaped so the compact Qd/Qa psums land
            # at their interleaved feature columns via strided DVE outs.
            tmp3 = statsp.tile([K, NQ, FQ], F32)
            var3 = statsp.tile([K, NQ, FQ], F32)
            tmp = chain[:, 4 * F_LOC:5 * F_LOC]
            std = chain[:, 6 * F_LOC:7 * F_LOC]
            inv = chain[:, 7 * F_LOC:8 * F_LOC]
            scal = chain[:, 8 * F_LOC:9 * F_LOC]
            shft = chain[:, 9 * F_LOC:10 * F_LOC]
            nc.scalar.activation(tmp3[:], mean_ps[:], AFT.Square)
            nc.vector.tensor_sub(var3[:, :, 0:2], ps_Qd[:], tmp3[:, :, 0:2])
            nc.vector.tensor_sub(var3[:, :, 2:3], ps_Qa[:], tmp3[:, :, 2:3])
            nc.vector.tensor_sub(var3[:, :, 3:4], msqz_ps[:, :, 3:4],
                                 tmp3[:, :, 3:4])
            nc.scalar.activation(std, var3[:], AFT.Sqrt, bias=epst)
            nc.vector.reciprocal(inv, std)
            nc.vector.tensor_mul(scal, inv, wt)
            nc.vector.tensor_mul(tmp, mean_ps[:], scal)
            nc.vector.tensor_sub(shft, bt, tmp)

            # ---- select per-batch scale/shift: [128, 32] per block ----
            selps0 = psum.tile([BB, 2 * F_LOC], F32)
            selps1 = psum.tile([BB, 2 * F_LOC], F32)
            for bb in range(NB):
                sp = selps0 if bb % 2 == 0 else selps1
                nc.tensor.matmul(sp[:], maskTt[:, bb * BB:(bb + 1) * BB],
                                 chain[:, 8 * F_LOC:10 * F_LOC],
                                 start=True, stop=True)
                nc.vector.tensor_copy(
                    sel[:, bb * 2 * F_LOC:(bb + 1) * 2 * F_LOC], sp[:])

            # ---- pass 2: y = x*scale + shift from resident tiles ----
            # scalar HWDGE queue for stores: spreads each one across all 16
            # DMA channels (a gpsimd-issued store runs on ONE channel at
            # ~26 GB/s, 39 us per tile).  Tile (0,0) is all-DVE since ACT
            # is still finishing the chain then.
            op_idx = 0
            for bb in range(NB):
                for q in range(NQ):
                    xt = res_tiles[(bb, q)]
                    yt = yout.tile([BB, FQ, L], BF16, tag="ys")
                    for j in range(FQ):
                        f = q * FQ + j
                        sc = sel[:, bb * 2 * F_LOC + f:
                                 bb * 2 * F_LOC + f + 1]
                        sh = sel[:, bb * 2 * F_LOC + F_LOC + f:
                                 bb * 2 * F_LOC + F_LOC + f + 1]
                        if op_idx % 4 == 3 and (bb, q) != (0, 0):
                            nc.scalar.activation(yt[:, j], xt[:, j],
                                                 AFT.Identity,
                                                 bias=sh, scale=sc)
                        else:
                            nc.vector.tensor_scalar(yt[:, j], xt[:, j],
                                                    sc, sh,
                                                    ALU.mult, ALU.add)
                        op_idx += 1
                    nc.scalar.dma_start(
                        y[bb * BB:(bb + 1) * BB, q * FQ:(q + 1) * FQ, :],
                        yt[:])

    nc.finalize()
    return nc


def _get_nc():
    global _built
    if _built is None:
        _built = _build()
    return _built


def _host_inputs(x, labels, weight, bias):
    labels = np.asarray(labels).astype(np.int64)
    counts = np.bincount(labels, minlength=K).astype(np.float64) * L
    rcp = (1.0 / np.maximum(counts, 1.0)).astype(np.float32).reshape(K, 1)

    maskb = np.zeros((BB, NB * K + F_LOC + K), dtype=np.float32)
    maskTb = np.zeros((K, B), dtype=np.float32)
    mexp = np.zeros((BB, NB * F_LOC * BB), dtype=np.float32)
    p = np.arange(BB)
    rcpf = rcp.reshape(K)
    for bb in range(NB):
        lab = labels[bb * BB:(bb + 1) * BB]
        maskb[p, bb * K + lab] = rcpf[lab]       # rcp folded: Q matmul -> E[x^2]
        for f in range(F_LOC):
            mexp[p, (bb * F_LOC + f) * BB + 8 * f + lab] = 1.0
    maskb[p, NB * K + p // K] = 1.0              # E16: p//8 == f
    maskb[p, NB * K + F_LOC + p % K] = rcpf[p % K]   # P8*rcp: rearrange -> mean
    maskTb[labels, np.arange(B)] = 1.0
    mexp8 = mexp.astype(mybir.dt.np(FP8))

    xb = np.ascontiguousarray(x).astype(ml_dtypes.bfloat16)
    in_maps = []
    for c in range(N_CORES):
        in_maps.append({
            "x": np.ascontiguousarray(xb[:, c * F_LOC:(c + 1) * F_LOC, :]),
            "mexp": mexp8,
            "maskb": maskb,
            "maskTb": maskTb,
            "rcp_cnt": rcp,
            "epsv": np.full((K, 1), EPS, dtype=np.float32),
            "wloc": np.ascontiguousarray(
                weight[:, c * F_LOC:(c + 1) * F_LOC].astype(np.float32)),
            "bloc": np.ascontiguousarray(
                bias[:, c * F_LOC:(c + 1) * F_LOC].astype(np.float32)),
        })
    return in_maps


def run(x, labels, weight, bias, trace=False):
    nc = _get_nc()
    in_maps = _host_inputs(x, labels, weight, bias)
    res = bass_utils.run_bass_kernel_spmd(nc, in_maps, list(range(N_CORES)),
                                          trace=trace)
    out = np.concatenate(
        [res.results[c]["y"] for c in range(N_CORES)],
        axis=1).astype(np.float32)
    return out, res


def kernel(x, labels, weight, bias):
    out, _ = run(np.asarray(x, dtype=np.float32), labels,
                 np.asarray(weight, dtype=np.float32),
                 np.asarray(bias, dtype=np.float32))
    return out
